# revision 18
# baseline (speedup 1.0000x reference)
"""Trainium2 Bass kernel: contrastive loss with negative mining.

Math (see module docstring of the original problem):
    centers  = mean over contiguous chunks of 8 rows               [n/8, d]
    x_pos    = x + 0.5*(center - x)        => |x - x_pos| = 0.5*|x - center|
    sim      = x @ x.T                                             [n, n]
    neg_idx  = argmax_j sim[i, j] excluding j in i's group-of-4
    d_ap     = mean_d |x - x_pos|,  d_an = mean_d |x - x_neg|
    loss     = sum( (1/8) * d_ap / (d_an + 1e-7) )

Distribution: data-parallel over rows, 8 NeuronCores, 1024 rows each.
Every core receives the full x (and a bf16 x.T) in its own DRAM, so no
collectives are needed; per-row losses are returned and summed on host.

Per core:
  - sim rows computed as bf16 matmuls (stationary = xT slice, moving = xT)
    in 512-wide column strips, f32 PSUM accumulation over 16 k-blocks.
  - Each strip tile [128, 512] is evacuated to SBUF as bf16 (ScalarE), then
    DVE max/max_index extract the top-8 values + indices per row.
  - The group-of-4 exclusion is applied on the candidate lists: a row's
    excluded group spans at most 4 of a strip's top-8, so the best valid
    candidate always survives.  Candidates are masked via integer compares
    against per-partition group bounds, then reduced to the argmax index.
  - x_neg rows are gathered from DRAM with a GPSIMD indirect DMA.
  - d_ap uses y = (I - blockdiag(ones(8,8)/8)) @ x_tile (fp32 matmul) and a
    ScalarE Abs+accumulate; d_an uses DVE subtract + abs-reduce.
"""

import math

import ml_dtypes
import numpy as np

import concourse.bass as bass
import concourse.mybir as mybir
import concourse.tile as tile
from concourse import bacc
from concourse.bass import IndirectOffsetOnAxis
from concourse.bass_utils import run_bass_kernel_spmd

BF16 = mybir.dt.bfloat16
F32 = mybir.dt.float32
U32 = mybir.dt.uint32
ALU = mybir.AluOpType
ACTF = mybir.ActivationFunctionType
AXX = mybir.AxisListType.X

P = 128         # partitions / row-tile height
JS = 512        # similarity column-strip width
CHUNK = 8       # rows averaged per center
GROUP = 4       # negative-mining exclusion window
WEIGHT = 1.0 / 8
EPS = 1e-7
NEG_BIG = -1e30


class Cfg:
    def __init__(self, n=8192, d=2048, cores=8, fp8=False):
        self.n, self.d, self.cores, self.fp8 = n, d, cores, fp8
        self.r = n // cores            # rows per core
        self.it = self.r // P          # i-tiles per core
        self.nj = n // JS              # column strips
        self.kb = d // P               # contraction blocks
        self.ch = max(1, d // JS)      # 512-wide d-chunks for the d_ap matmul
        self.cw = min(d, JS)           # chunk width
        assert n % (cores * P) == 0 and d % P == 0 and n % JS == 0
        assert d % self.cw == 0


def _body(tc: tile.TileContext, cfg: Cfg, io: dict):
    nc = tc.nc
    ctxpools = {}

    def pool(name, bufs, space="SBUF"):
        if name not in ctxpools:
            ctxpools[name] = tc.alloc_tile_pool(name=name, bufs=bufs, space=space)
        return ctxpools[name]

    consts = pool("consts", 1)
    m2_sb = consts.tile_from(io["m2"])                       # [128,128] f32
    g0_sb = consts.tile_from(io["g0f"])                      # [128,IT] f32
    g3_sb = consts.tile_from(io["g3f"])                      # [128,IT] f32
    offs_sb = consts.tile_from(io["offsf"])                  # [128,NJ*8] f32

    sim_dt = mybir.dt.float8e4 if cfg.fp8 else BF16

    # resident stationary xT slice: [128, KB*R], k-block major
    xs_sb = pool("xs", 1).tile([P, cfg.kb * cfg.r], sim_dt, name="xs_sb")
    nc.sync.dma_start(
        out=xs_sb[:].rearrange("p (a r) -> p a r", a=cfg.kb),
        in_=io["xs"][:, :].rearrange("(a p) r -> p a r", p=P),
    )

    # resident x rows (this core's shard): [128, IT*D] f32, i-tile major
    xr_sb = pool("xr", 1).tile([P, cfg.it * cfg.d], F32, name="xr_sb")
    nc.sync.dma_start(
        out=xr_sb[:].rearrange("p (a d) -> p a d", a=cfg.it),
        in_=io["xr"][:, :].rearrange("(a p) d -> p a d", p=P),
    )

    psum = pool("ps", 8, space="PSUM")
    small = pool("small", 1)
    sap = small.tile([P, cfg.it * cfg.ch], F32, name="sap")    # sum|y| per chunk
    san = small.tile([P, cfg.it], F32, name="san")             # sum|x-xneg|
    idxall = small.tile([P, cfg.it], U32, name="idxall")       # neg indices
    cv_sb = small.tile([P, cfg.it * cfg.nj * 8], F32, name="cv_sb")    # top8 vals
    ci_sb = small.tile([P, cfg.it * cfg.nj * 8], U32, name="ci_sb")    # top8 idxs

    yabs = pool("yabs", 2)

    # ---- Phase A: d_ap = sum_d |y|, y = M2 @ x_tile  (fp32 matmuls) ----
    for it in range(cfg.it):
        for c in range(cfg.ch):
            ps_y = psum.tile([P, cfg.cw], F32, name="ps_y", tag="ps")
            nc.tensor.matmul(
                out=ps_y[:],
                lhsT=m2_sb[:],
                rhs=xr_sb[:, it * cfg.d + c * cfg.cw: it * cfg.d + (c + 1) * cfg.cw],
                start=True, stop=True,
            )
            y_sc = yabs.tile([P, cfg.cw], F32, name="y_sc")
            nc.scalar.activation(
                out=y_sc[:], in_=ps_y[:], func=ACTF.Abs,
                accum_out=sap[:, it * cfg.ch + c: it * cfg.ch + c + 1],
            )

    # ---- Phase B: sim strips + per-strip top-8 ----
    xmp = pool("xm", 2)
    evac = pool("evac", 4)
    for j in range(cfg.nj):
        xm_sb = xmp.tile([P, cfg.kb * JS], sim_dt, name="xm_sb")
        nc.sync.dma_start(
            out=xm_sb[:].rearrange("p (a b) -> p a b", a=cfg.kb),
            in_=io["xm"][:, j * JS:(j + 1) * JS].rearrange("(a p) b -> p a b", p=P),
        )
        xs3 = xs_sb[:].rearrange("p (a r) -> p a r", a=cfg.kb)
        xm3 = xm_sb[:].rearrange("p (a b) -> p a b", a=cfg.kb)
        for it in range(cfg.it):
            ps_s = psum.tile([P, JS], F32, name="ps_s", tag="ps")
            if cfg.fp8:
                for k in range(0, cfg.kb, 2):
                    nc.tensor.matmul(
                        out=ps_s[:],
                        lhsT=xs3[:, k:k + 2, it * P:(it + 1) * P],
                        rhs=xm3[:, k:k + 2, :],
                        start=(k == 0), stop=(k == cfg.kb - 2),
                        perf_mode=mybir.MatmulPerfMode.DoubleRow,
                    )
            else:
                for k in range(cfg.kb):
                    nc.tensor.matmul(
                        out=ps_s[:],
                        lhsT=xs_sb[:, k * cfg.r + it * P: k * cfg.r + (it + 1) * P],
                        rhs=xm_sb[:, k * JS:(k + 1) * JS],
                        start=(k == 0), stop=(k == cfg.kb - 1),
                    )
            sstrip = evac.tile([P, JS], F32, name="sstrip")
            nc.scalar.copy(out=sstrip[:], in_=ps_s[:])
            q0 = (it * cfg.nj + j) * 8
            nc.vector.max(out=cv_sb[:, q0:q0 + 8], in_=sstrip[:])
            nc.vector.max_index(
                out=ci_sb[:, q0:q0 + 8],
                in_max=cv_sb[:, q0:q0 + 8],
                in_values=sstrip[:],
            )

    # ---- Phase C: combine candidates, gather x_neg, d_an, loss ----
    comb = pool("comb", 2)
    xneg_p = pool("xneg", 2)
    diff_p = pool("diff", 2)
    nq = cfg.nj * 8
    for it in range(cfg.it):
        cv_i = cv_sb[:, it * nq:(it + 1) * nq]
        ci_i = ci_sb[:, it * nq:(it + 1) * nq]
        cif = comb.tile([P, nq], F32, name="cif")
        nc.vector.tensor_copy(out=cif[:], in_=ci_i)
        gidx = comb.tile([P, nq], F32, name="gidx")
        nc.vector.tensor_tensor(out=gidx[:], in0=cif[:], in1=offs_sb[:], op=ALU.add)
        b1 = comb.tile([P, nq], F32, name="b1")
        nc.vector.tensor_scalar(
            out=b1[:], in0=gidx[:], scalar1=g0_sb[:, it:it + 1], scalar2=None,
            op0=ALU.is_ge,
        )
        b2 = comb.tile([P, nq], F32, name="b2")
        nc.vector.tensor_scalar(
            out=b2[:], in0=gidx[:], scalar1=g3_sb[:, it:it + 1], scalar2=None,
            op0=ALU.is_le,
        )
        # mv = cv + (b1 * NEG_BIG) * b2   (masked candidate values)
        msk = comb.tile([P, nq], F32, name="msk")
        nc.vector.scalar_tensor_tensor(
            out=msk[:], in0=b1[:], scalar=NEG_BIG, in1=b2[:],
            op0=ALU.mult, op1=ALU.mult,
        )
        mv = comb.tile([P, nq], F32, name="mv")
        nc.vector.tensor_tensor(out=mv[:], in0=cv_i, in1=msk[:], op=ALU.add)
        mx = comb.tile([P, 1], F32, name="mx")
        nc.vector.tensor_reduce(out=mx[:], in_=mv[:], axis=AXX, op=ALU.max)
        sel = comb.tile([P, nq], F32, name="sel")
        nc.vector.tensor_scalar(
            out=sel[:], in0=mv[:], scalar1=mx[:], scalar2=None, op0=ALU.is_ge,
        )
        pick = comb.tile([P, nq], F32, name="pick")
        nc.vector.tensor_tensor(out=pick[:], in0=sel[:], in1=gidx[:], op=ALU.mult)
        idxf = comb.tile([P, 1], F32, name="idxf")
        nc.vector.tensor_reduce(out=idxf[:], in_=pick[:], axis=AXX, op=ALU.max)
        nc.vector.tensor_copy(out=idxall[:, it:it + 1], in_=idxf[:])

        xneg = xneg_p.tile([P, cfg.d], F32, name="xneg")
        nc.gpsimd.indirect_dma_start(
            out=xneg[:], out_offset=None,
            in_=io["xf"][:, :],
            in_offset=IndirectOffsetOnAxis(ap=idxall[:, it:it + 1], axis=0),
        )
        diff = diff_p.tile([P, cfg.d], F32, name="diff")
        nc.vector.tensor_tensor(
            out=diff[:], in0=xr_sb[:, it * cfg.d:(it + 1) * cfg.d], in1=xneg[:],
            op=ALU.subtract,
        )
        nc.vector.tensor_reduce(
            out=san[:, it:it + 1], in_=diff[:], axis=AXX, op=ALU.add,
            apply_absolute_value=True,
        )

    # ---- Final: per-row loss ----
    fin = pool("fin", 1)
    sap8 = fin.tile([P, cfg.it], F32, name="sap8")
    sap3 = sap[:].rearrange("p (a b) -> p a b", a=cfg.it)
    nc.vector.tensor_reduce(out=sap8[:], in_=sap3, axis=AXX, op=ALU.add)
    t1 = fin.tile([P, cfg.it], F32, name="t1")
    nc.vector.tensor_scalar(
        out=t1[:], in0=san[:], scalar1=1.0 / cfg.d, scalar2=EPS,
        op0=ALU.mult, op1=ALU.add,
    )
    rec = fin.tile([P, cfg.it], F32, name="rec")
    nc.vector.reciprocal(out=rec[:], in_=t1[:])
    t2 = fin.tile([P, cfg.it], F32, name="t2")
    nc.vector.tensor_tensor(out=t2[:], in0=sap8[:], in1=rec[:], op=ALU.mult)
    lossv = fin.tile([P, cfg.it], F32, name="lossv")
    nc.vector.tensor_scalar(
        out=lossv[:], in0=t2[:], scalar1=0.5 * WEIGHT / cfg.d, scalar2=None,
        op0=ALU.mult,
    )
    nc.sync.dma_start(out=io["loss_part"][:, :], in_=lossv[:])
    nc.sync.dma_start(out=io["nidx"][:, :], in_=idxall[:])

    for p in reversed(list(ctxpools.values())):
        p.release()


def build(cfg: Cfg) -> bass.Bass:
    nc = bacc.Bacc("TRN2", target_bir_lowering=False, debug=False)
    sim_dt = mybir.dt.float8e4 if cfg.fp8 else BF16
    io = {
        "xm": nc.dram_tensor("xm", [cfg.d, cfg.n], sim_dt, kind="ExternalInput").ap(),
        "xs": nc.dram_tensor("xs", [cfg.d, cfg.r], sim_dt, kind="ExternalInput").ap(),
        "xr": nc.dram_tensor("xr", [cfg.r, cfg.d], F32, kind="ExternalInput").ap(),
        "xf": nc.dram_tensor("xf", [cfg.n, cfg.d], F32, kind="ExternalInput").ap(),
        "m2": nc.dram_tensor("m2", [P, P], F32, kind="ExternalInput").ap(),
        "offsf": nc.dram_tensor("offsf", [P, cfg.nj * 8], F32, kind="ExternalInput").ap(),
        "g0f": nc.dram_tensor("g0f", [P, cfg.it], F32, kind="ExternalInput").ap(),
        "g3f": nc.dram_tensor("g3f", [P, cfg.it], F32, kind="ExternalInput").ap(),
        "loss_part": nc.dram_tensor("loss_part", [P, cfg.it], F32, kind="ExternalOutput").ap(),
        "nidx": nc.dram_tensor("nidx", [P, cfg.it], U32, kind="ExternalOutput").ap(),
    }
    with tile.TileContext(nc) as tc:
        _body(tc, cfg, io)
    nc.compile()
    return nc


def make_in_maps(cfg: Cfg, x: np.ndarray) -> list[dict]:
    x = np.ascontiguousarray(x, dtype=np.float32)
    sim_np = ml_dtypes.float8_e4m3 if cfg.fp8 else ml_dtypes.bfloat16
    xt_bf = np.ascontiguousarray(x.T.astype(sim_np))

    m2 = np.eye(P, dtype=np.float32)
    for c in range(P // CHUNK):
        m2[c * CHUNK:(c + 1) * CHUNK, c * CHUNK:(c + 1) * CHUNK] -= 1.0 / CHUNK

    offsf = np.zeros((P, cfg.nj * 8), dtype=np.float32)
    for j in range(cfg.nj):
        offsf[:, j * 8:(j + 1) * 8] = j * JS

    pvec = np.arange(P, dtype=np.float32)
    in_maps = []
    for c in range(cfg.cores):
        g0 = np.zeros((P, cfg.it), dtype=np.float32)
        for it in range(cfg.it):
            g0[:, it] = c * cfg.r + it * P + (pvec // GROUP) * GROUP
        in_maps.append({
            "xm": xt_bf,
            "xs": np.ascontiguousarray(xt_bf[:, c * cfg.r:(c + 1) * cfg.r]),
            "xr": np.ascontiguousarray(x[c * cfg.r:(c + 1) * cfg.r]),
            "xf": x,
            "m2": m2,
            "offsf": offsf,
            "g0f": g0,
            "g3f": g0 + (GROUP - 1),
        })
    return in_maps


def reduce_outputs(cfg: Cfg, results: list[dict]) -> np.ndarray:
    total = 0.0
    for res in results:
        total += float(res["loss_part"].astype(np.float64).sum())
    return np.float32(total)


def run(cfg: Cfg, x: np.ndarray, trace: bool = False):
    nc = build(cfg)
    in_maps = make_in_maps(cfg, x)
    out = run_bass_kernel_spmd(nc, in_maps, list(range(cfg.cores)), trace=trace)
    return out


def kernel(x: np.ndarray) -> np.ndarray:
    cfg = Cfg(n=8192, d=2048, cores=8)
    out = run(cfg, x)
    return reduce_outputs(cfg, out.results)


# revision 23
# speedup vs baseline: 1.7790x; 1.7790x over previous
"""Trainium2 Bass kernel: contrastive loss with negative mining.

Math (see module docstring of the original problem):
    centers  = mean over contiguous chunks of 8 rows               [n/8, d]
    x_pos    = x + 0.5*(center - x)        => |x - x_pos| = 0.5*|x - center|
    sim      = x @ x.T                                             [n, n]
    neg_idx  = argmax_j sim[i, j] excluding j in i's group-of-4
    d_ap     = mean_d |x - x_pos|,  d_an = mean_d |x - x_neg|
    loss     = sum( (1/8) * d_ap / (d_an + 1e-7) )

Distribution: data-parallel over rows, 8 NeuronCores, 1024 rows each.
Every core receives the full x (and a bf16 x.T) in its own DRAM, so no
collectives are needed; per-row losses are returned and summed on host.

Per core:
  - sim rows computed as bf16 matmuls (stationary = xT slice, moving = xT)
    in 512-wide column strips, f32 PSUM accumulation over 16 k-blocks.
  - Each strip tile [128, 512] is evacuated to SBUF as bf16 (ScalarE), then
    DVE max/max_index extract the top-8 values + indices per row.
  - The group-of-4 exclusion is applied on the candidate lists: a row's
    excluded group spans at most 4 of a strip's top-8, so the best valid
    candidate always survives.  Candidates are masked via integer compares
    against per-partition group bounds, then reduced to the argmax index.
  - x_neg rows are gathered from DRAM with a GPSIMD indirect DMA.
  - d_ap uses y = (I - blockdiag(ones(8,8)/8)) @ x_tile (fp32 matmul) and a
    ScalarE Abs+accumulate; d_an uses DVE subtract + abs-reduce.
"""

import math

import ml_dtypes
import numpy as np

import concourse.bass as bass
import concourse.mybir as mybir
import concourse.tile as tile
from concourse import bacc
from concourse.bass import IndirectOffsetOnAxis
from concourse.bass_utils import run_bass_kernel_spmd

BF16 = mybir.dt.bfloat16
F32 = mybir.dt.float32
U32 = mybir.dt.uint32
ALU = mybir.AluOpType
ACTF = mybir.ActivationFunctionType
AXX = mybir.AxisListType.X

P = 128         # partitions / row-tile height
JS = 512        # similarity column-strip width
CHUNK = 8       # rows averaged per center
GROUP = 4       # negative-mining exclusion window
WEIGHT = 1.0 / 8
EPS = 1e-7
NEG_BIG = -1e30


class Cfg:
    def __init__(self, n=8192, d=2048, cores=8, fp8=False):
        self.n, self.d, self.cores, self.fp8 = n, d, cores, fp8
        self.r = n // cores            # rows per core
        self.it = self.r // P          # i-tiles per core
        self.nj = n // JS              # column strips
        self.kb = d // P               # contraction blocks
        self.ch = max(1, d // JS)      # 512-wide d-chunks for the d_ap matmul
        self.cw = min(d, JS)           # chunk width
        assert n % (cores * P) == 0 and d % P == 0 and n % JS == 0
        assert d % self.cw == 0


def _body(tc: tile.TileContext, cfg: Cfg, io: dict):
    nc = tc.nc
    ctxpools = {}

    def pool(name, bufs, space="SBUF"):
        if name not in ctxpools:
            ctxpools[name] = tc.alloc_tile_pool(name=name, bufs=bufs, space=space)
        return ctxpools[name]

    consts = pool("consts", 1)
    m2_sb = consts.tile_from(io["m2"])                       # [128,128] f32
    g0_sb = consts.tile_from(io["g0f"])                      # [128,IT] f32
    g3_sb = consts.tile_from(io["g3f"])                      # [128,IT] f32
    offs_sb = consts.tile_from(io["offsf"])                  # [128,NJ*8] f32

    sim_dt = mybir.dt.float8e4 if cfg.fp8 else BF16

    # resident stationary xT slice: [128, KB*R], k-block major
    xs_sb = pool("xs", 1).tile([P, cfg.kb * cfg.r], sim_dt, name="xs_sb")
    nc.sync.dma_start(
        out=xs_sb[:].rearrange("p (a r) -> p a r", a=cfg.kb),
        in_=io["xs"][:, :].rearrange("(a p) r -> p a r", p=P),
    )

    psum = pool("ps", 8, space="PSUM")
    small = pool("small", 1)
    sap = small.tile([P, cfg.it * cfg.ch], F32, name="sap")    # sum|y| per chunk
    san = small.tile([P, cfg.it], F32, name="san")             # sum|x-xneg|
    idxall = small.tile([P, cfg.it], U32, name="idxall")       # neg indices
    cv_sb = small.tile([P, cfg.it * cfg.nj * 8], BF16, name="cv_sb")   # top8 vals
    ci_sb = small.tile([P, cfg.it * cfg.nj * 8], U32, name="ci_sb")    # top8 idxs

    yabs = pool("yabs", 2)

    # ---- Phase B: sim strips + per-strip top-8 ----
    xmp = pool("xm", 2)
    evac = pool("evac", 4)
    for j in range(cfg.nj):
        xm_sb = xmp.tile([P, cfg.kb * JS], sim_dt, name="xm_sb")
        nc.sync.dma_start(
            out=xm_sb[:].rearrange("p (a b) -> p a b", a=cfg.kb),
            in_=io["xm"][:, j * JS:(j + 1) * JS].rearrange("(a p) b -> p a b", p=P),
        )
        xs3 = xs_sb[:].rearrange("p (a r) -> p a r", a=cfg.kb)
        xm3 = xm_sb[:].rearrange("p (a b) -> p a b", a=cfg.kb)
        for it in range(cfg.it):
            ps_s = psum.tile([P, JS], F32, name="ps_s", tag="ps")
            if cfg.fp8:
                for k in range(0, cfg.kb, 2):
                    nc.tensor.matmul(
                        out=ps_s[:],
                        lhsT=xs3[:, k:k + 2, it * P:(it + 1) * P],
                        rhs=xm3[:, k:k + 2, :],
                        start=(k == 0), stop=(k == cfg.kb - 2),
                        perf_mode=mybir.MatmulPerfMode.DoubleRow,
                    )
            else:
                for k in range(cfg.kb):
                    nc.tensor.matmul(
                        out=ps_s[:],
                        lhsT=xs_sb[:, k * cfg.r + it * P: k * cfg.r + (it + 1) * P],
                        rhs=xm_sb[:, k * JS:(k + 1) * JS],
                        start=(k == 0), stop=(k == cfg.kb - 1),
                    )
            sstrip = evac.tile([P, JS], BF16, name="sstrip")
            nc.scalar.copy(out=sstrip[:], in_=ps_s[:])
            q0 = (it * cfg.nj + j) * 8
            nc.vector.max(out=cv_sb[:, q0:q0 + 8], in_=sstrip[:])
            nc.vector.max_index(
                out=ci_sb[:, q0:q0 + 8],
                in_max=cv_sb[:, q0:q0 + 8],
                in_values=sstrip[:],
            )

    # ---- Phase A (emitted after B so PE starts on sim immediately):
    #      d_ap = sum_d |y|,  y = M2 @ x_tile  (fp32 matmuls, overlap C) ----
    xr_sb = pool("xr", 1).tile([P, cfg.it * cfg.d], F32, name="xr_sb")
    nc.sync.dma_start(
        out=xr_sb[:].rearrange("p (a d) -> p a d", a=cfg.it),
        in_=io["xr"][:, :].rearrange("(a p) d -> p a d", p=P),
    )
    for it in range(cfg.it):
        for c in range(cfg.ch):
            ps_y = psum.tile([P, cfg.cw], F32, name="ps_y", tag="ps")
            nc.tensor.matmul(
                out=ps_y[:],
                lhsT=m2_sb[:],
                rhs=xr_sb[:, it * cfg.d + c * cfg.cw: it * cfg.d + (c + 1) * cfg.cw],
                start=True, stop=True,
            )
            y_sc = yabs.tile([P, cfg.cw], F32, name="y_sc")
            nc.scalar.activation(
                out=y_sc[:], in_=ps_y[:], func=ACTF.Abs,
                accum_out=sap[:, it * cfg.ch + c: it * cfg.ch + c + 1],
            )

    # ---- Phase C: combine candidates, gather x_neg, d_an, loss ----
    comb = pool("comb", 2)
    xneg_p = pool("xneg", 2)
    diff_p = pool("diff", 2)
    nq = cfg.nj * 8
    for it in range(cfg.it):
        cv_i = cv_sb[:, it * nq:(it + 1) * nq]
        ci_i = ci_sb[:, it * nq:(it + 1) * nq]
        cif = comb.tile([P, nq], F32, name="cif")
        nc.vector.tensor_copy(out=cif[:], in_=ci_i)
        gidx = comb.tile([P, nq], F32, name="gidx")
        nc.vector.tensor_tensor(out=gidx[:], in0=cif[:], in1=offs_sb[:], op=ALU.add)
        b1 = comb.tile([P, nq], F32, name="b1")
        nc.vector.tensor_scalar(
            out=b1[:], in0=gidx[:], scalar1=g0_sb[:, it:it + 1], scalar2=None,
            op0=ALU.is_ge,
        )
        b2 = comb.tile([P, nq], F32, name="b2")
        nc.vector.tensor_scalar(
            out=b2[:], in0=gidx[:], scalar1=g3_sb[:, it:it + 1], scalar2=None,
            op0=ALU.is_le,
        )
        # mv = cv + (b1 * NEG_BIG) * b2   (masked candidate values)
        msk = comb.tile([P, nq], F32, name="msk")
        nc.vector.scalar_tensor_tensor(
            out=msk[:], in0=b1[:], scalar=NEG_BIG, in1=b2[:],
            op0=ALU.mult, op1=ALU.mult,
        )
        cvf = comb.tile([P, nq], F32, name="cvf")
        nc.vector.tensor_copy(out=cvf[:], in_=cv_i)
        mv = comb.tile([P, nq], F32, name="mv")
        nc.vector.tensor_tensor(out=mv[:], in0=cvf[:], in1=msk[:], op=ALU.add)
        mx = comb.tile([P, 1], F32, name="mx")
        nc.vector.tensor_reduce(out=mx[:], in_=mv[:], axis=AXX, op=ALU.max)
        sel = comb.tile([P, nq], F32, name="sel")
        nc.vector.tensor_scalar(
            out=sel[:], in0=mv[:], scalar1=mx[:], scalar2=None, op0=ALU.is_ge,
        )
        pick = comb.tile([P, nq], F32, name="pick")
        nc.vector.tensor_tensor(out=pick[:], in0=sel[:], in1=gidx[:], op=ALU.mult)
        idxf = comb.tile([P, 1], F32, name="idxf")
        nc.vector.tensor_reduce(out=idxf[:], in_=pick[:], axis=AXX, op=ALU.max)
        nc.vector.tensor_copy(out=idxall[:, it:it + 1], in_=idxf[:])

        xneg = xneg_p.tile([P, cfg.d], F32, name="xneg")
        nc.gpsimd.indirect_dma_start(
            out=xneg[:], out_offset=None,
            in_=io["xf"][:, :],
            in_offset=IndirectOffsetOnAxis(ap=idxall[:, it:it + 1], axis=0),
        )
        diff = diff_p.tile([P, cfg.d], F32, name="diff")
        nc.vector.tensor_tensor(
            out=diff[:], in0=xr_sb[:, it * cfg.d:(it + 1) * cfg.d], in1=xneg[:],
            op=ALU.subtract,
        )
        nc.vector.tensor_reduce(
            out=san[:, it:it + 1], in_=diff[:], axis=AXX, op=ALU.add,
            apply_absolute_value=True,
        )

    # ---- Final: per-row loss ----
    fin = pool("fin", 1)
    sap8 = fin.tile([P, cfg.it], F32, name="sap8")
    sap3 = sap[:].rearrange("p (a b) -> p a b", a=cfg.it)
    nc.vector.tensor_reduce(out=sap8[:], in_=sap3, axis=AXX, op=ALU.add)
    t1 = fin.tile([P, cfg.it], F32, name="t1")
    nc.vector.tensor_scalar(
        out=t1[:], in0=san[:], scalar1=1.0 / cfg.d, scalar2=EPS,
        op0=ALU.mult, op1=ALU.add,
    )
    rec = fin.tile([P, cfg.it], F32, name="rec")
    nc.vector.reciprocal(out=rec[:], in_=t1[:])
    t2 = fin.tile([P, cfg.it], F32, name="t2")
    nc.vector.tensor_tensor(out=t2[:], in0=sap8[:], in1=rec[:], op=ALU.mult)
    lossv = fin.tile([P, cfg.it], F32, name="lossv")
    nc.vector.tensor_scalar(
        out=lossv[:], in0=t2[:], scalar1=0.5 * WEIGHT / cfg.d, scalar2=None,
        op0=ALU.mult,
    )
    nc.sync.dma_start(out=io["loss_part"][:, :], in_=lossv[:])
    nc.sync.dma_start(out=io["nidx"][:, :], in_=idxall[:])

    for p in reversed(list(ctxpools.values())):
        p.release()


def build(cfg: Cfg) -> bass.Bass:
    nc = bacc.Bacc("TRN2", target_bir_lowering=False, debug=False)
    sim_dt = mybir.dt.float8e4 if cfg.fp8 else BF16
    io = {
        "xm": nc.dram_tensor("xm", [cfg.d, cfg.n], sim_dt, kind="ExternalInput").ap(),
        "xs": nc.dram_tensor("xs", [cfg.d, cfg.r], sim_dt, kind="ExternalInput").ap(),
        "xr": nc.dram_tensor("xr", [cfg.r, cfg.d], F32, kind="ExternalInput").ap(),
        "xf": nc.dram_tensor("xf", [cfg.n, cfg.d], F32, kind="ExternalInput").ap(),
        "m2": nc.dram_tensor("m2", [P, P], F32, kind="ExternalInput").ap(),
        "offsf": nc.dram_tensor("offsf", [P, cfg.nj * 8], F32, kind="ExternalInput").ap(),
        "g0f": nc.dram_tensor("g0f", [P, cfg.it], F32, kind="ExternalInput").ap(),
        "g3f": nc.dram_tensor("g3f", [P, cfg.it], F32, kind="ExternalInput").ap(),
        "loss_part": nc.dram_tensor("loss_part", [P, cfg.it], F32, kind="ExternalOutput").ap(),
        "nidx": nc.dram_tensor("nidx", [P, cfg.it], U32, kind="ExternalOutput").ap(),
    }
    with tile.TileContext(nc) as tc:
        _body(tc, cfg, io)
    nc.compile()
    return nc


def make_in_maps(cfg: Cfg, x: np.ndarray) -> list[dict]:
    x = np.ascontiguousarray(x, dtype=np.float32)
    sim_np = ml_dtypes.float8_e4m3 if cfg.fp8 else ml_dtypes.bfloat16
    xt_bf = np.ascontiguousarray(x.T.astype(sim_np))

    m2 = np.eye(P, dtype=np.float32)
    for c in range(P // CHUNK):
        m2[c * CHUNK:(c + 1) * CHUNK, c * CHUNK:(c + 1) * CHUNK] -= 1.0 / CHUNK

    offsf = np.zeros((P, cfg.nj * 8), dtype=np.float32)
    for j in range(cfg.nj):
        offsf[:, j * 8:(j + 1) * 8] = j * JS

    pvec = np.arange(P, dtype=np.float32)
    in_maps = []
    for c in range(cfg.cores):
        g0 = np.zeros((P, cfg.it), dtype=np.float32)
        for it in range(cfg.it):
            g0[:, it] = c * cfg.r + it * P + (pvec // GROUP) * GROUP
        in_maps.append({
            "xm": xt_bf,
            "xs": np.ascontiguousarray(xt_bf[:, c * cfg.r:(c + 1) * cfg.r]),
            "xr": np.ascontiguousarray(x[c * cfg.r:(c + 1) * cfg.r]),
            "xf": x,
            "m2": m2,
            "offsf": offsf,
            "g0f": g0,
            "g3f": g0 + (GROUP - 1),
        })
    return in_maps


def reduce_outputs(cfg: Cfg, results: list[dict]) -> np.ndarray:
    total = 0.0
    for res in results:
        total += float(res["loss_part"].astype(np.float64).sum())
    return np.float32(total)


def run(cfg: Cfg, x: np.ndarray, trace: bool = False):
    nc = build(cfg)
    in_maps = make_in_maps(cfg, x)
    out = run_bass_kernel_spmd(nc, in_maps, list(range(cfg.cores)), trace=trace)
    return out


def kernel(x: np.ndarray) -> np.ndarray:
    cfg = Cfg(n=8192, d=2048, cores=8)
    out = run(cfg, x)
    return reduce_outputs(cfg, out.results)
